# revision 1
# baseline (speedup 1.0000x reference)
"""Trainium2 Bass kernel for NoTPAttention (dense transformer block:
fused QKV projection -> multi-head attention -> output projection).

Sharding (8 NeuronCores): core c handles batch b = c // 4 and the 4 heads
g = 4*(c % 4) .. 4*(c % 4)+3 (head-parallel tensor parallelism).  Each core
computes its heads' partial out-projection [S, H] in fp32; the host sums the
4 partials per batch and adds the (folded) biases.

Numerics: all matmuls run in bf16 with fp32 PSUM accumulation (measured
absmax-relative error vs the fp32 reference: ~3e-3).  Softmax is computed
without max-subtraction (scores are bounded, |s| < ~3.5) with the
normalization deferred to the attention *output* (16x less work than
normalizing probabilities):
    attnT[d, q] = (sum_k v[k, d] * exp(sT[k, q])) / (sum_k exp(sT[k, q]))
The denominator comes from a ones-matmul on the tensor engine which lands it
already broadcast across partitions.  The v-bias is dropped in-kernel: after
normalization it contributes exactly b_v to every row, so the host folds
w_out @ b_v into the output bias.

Layout notes: qT/kT/attnT live as [128 (head-dim), head, seq] so every
matmul contracts over a full 128-partition tile with no transposes anywhere.
The qkv weights share SBUF slots with the attention exp-buffers (tag "e"):
they are dead once the projections finish, exactly when the exp buffers
start rotating.
"""

import numpy as np
import ml_dtypes

B, S, H = 2, 2048, 2048
NH, HD = 16, 128
P = 128
HT = H // P            # 16 hidden-dim tiles
G = 4                  # heads per core
GH = G * HD            # 512: head-group width per core
SCALE = 1.0 / float(np.sqrt(HD))
N_CORES = 8
XC = 512               # phase-1 x streaming chunk (s elements)
QC = 512               # attention query chunk
KT = S // P            # 16 key tiles

_CACHE = {}


def _build():
    import concourse.mybir as mybir
    import concourse.tile as tile
    from concourse import bacc

    dt = mybir.dt
    Alu = mybir.AluOpType
    Act = mybir.ActivationFunctionType

    nc = bacc.Bacc("TRN2", target_bir_lowering=False, debug=False,
                   enable_asserts=False)

    xt_d = nc.dram_tensor("xt", [H, S], dt.bfloat16, kind="ExternalInput").ap()
    wqt_d = nc.dram_tensor("wqt", [H, GH], dt.bfloat16, kind="ExternalInput").ap()
    wkt_d = nc.dram_tensor("wkt", [H, GH], dt.bfloat16, kind="ExternalInput").ap()
    wvt_d = nc.dram_tensor("wvt", [H, GH], dt.bfloat16, kind="ExternalInput").ap()
    bqs_d = nc.dram_tensor("bqs", [P, G], dt.float32, kind="ExternalInput").ap()
    bk_d = nc.dram_tensor("bk", [P, G], dt.float32, kind="ExternalInput").ap()
    wot_d = nc.dram_tensor("wot", [GH, H], dt.bfloat16, kind="ExternalInput").ap()
    out_d = nc.dram_tensor("partial", [S, H], dt.float32, kind="ExternalOutput").ap()

    xt_r = xt_d.rearrange("(ht p) s -> p ht s", p=P)      # [128, 16, 2048]
    wqt_r = wqt_d.rearrange("(ht p) o -> p ht o", p=P)    # [128, 16, 512]
    wkt_r = wkt_d.rearrange("(ht p) o -> p ht o", p=P)
    wvt_r = wvt_d.rearrange("(ht p) o -> p ht o", p=P)
    wot_r = wot_d.rearrange("(g p) o -> p g o", p=P)      # [128, 4, 2048]

    NXC = S // XC      # 4
    NQC = S // QC      # 4

    with tile.TileContext(nc) as tc:
        with (
            tc.tile_pool(name="consts", bufs=1) as consts,
            tc.tile_pool(name="wpool", bufs=1) as wpool,
            tc.tile_pool(name="xpool", bufs=2) as xpool,
            tc.tile_pool(name="big", bufs=1) as big,
            tc.tile_pool(name="epool", bufs=4) as epool,
            tc.tile_pool(name="small", bufs=2) as small,
            tc.tile_pool(name="psum", bufs=2, space="PSUM") as psum,
        ):
            # --- startup DMAs, critical-path first and split so the first
            # q-matmuls (head 0/1, ht 0..7) can start after ~2MB of traffic ---
            wq_sb = epool.tile([P, HT, GH], dt.bfloat16, tag="e", name="wq_sb")
            nc.sync.dma_start(wq_sb[:, :, 0:2 * HD], wqt_r[:, :, 0:2 * HD])
            xt0_sb = xpool.tile([P, HT, XC], dt.bfloat16, tag="xt",
                                name="xt0_sb")
            for q4 in range(4):
                nc.sync.dma_start(xt0_sb[:, 4 * q4:4 * (q4 + 1), :],
                                  xt_r[:, 4 * q4:4 * (q4 + 1), 0:XC])
            nc.sync.dma_start(wq_sb[:, :, 2 * HD:], wqt_r[:, :, 2 * HD:])
            bqs_sb = consts.tile([P, G], dt.float32)
            nc.sync.dma_start(bqs_sb[:], bqs_d)
            bk_sb = consts.tile([P, G], dt.float32)
            nc.sync.dma_start(bk_sb[:], bk_d)
            ones_sb = consts.tile([P, P], dt.bfloat16)
            nc.vector.memset(ones_sb[:], 1.0)
            wk_sb = epool.tile([P, HT, GH], dt.bfloat16, tag="e", name="wk_sb")
            nc.sync.dma_start(wk_sb[:], wkt_r)
            wv_sb = epool.tile([P, HT, GH], dt.bfloat16, tag="e", name="wv_sb")
            nc.sync.dma_start(wv_sb[:], wvt_r)

            qt_sb = big.tile([P, G, S], dt.bfloat16)   # q^T, scale+bias applied
            kt_sb = big.tile([P, G, S], dt.bfloat16)   # k^T, bias applied
            v_sb = big.tile([P, KT, GH], dt.bfloat16)  # v natural [s, o]
            at_sb = big.tile([P, G, S], dt.bfloat16)   # attn output^T

            # ---------------- Phase 1: QKV projections ----------------
            for xc in range(NXC):
                if xc == 0:
                    xt_sb = xt0_sb
                else:
                    xt_sb = xpool.tile([P, HT, XC], dt.bfloat16, tag="xt",
                                       name="xt_sb")
                    nc.sync.dma_start(xt_sb[:], xt_r[:, :, xc * XC:(xc + 1) * XC])
                sl = slice(xc * XC, (xc + 1) * XC)
                for h in range(G):
                    psq = psum.tile([P, 512], dt.float32, tag="mm")
                    for ht in range(HT):
                        nc.tensor.matmul(psq,
                                         wq_sb[:, ht, h * HD:(h + 1) * HD],
                                         xt_sb[:, ht, :],
                                         start=(ht == 0), stop=(ht == HT - 1))
                    nc.vector.tensor_scalar(qt_sb[:, h, sl], psq,
                                            SCALE, bqs_sb[:, h:h + 1],
                                            Alu.mult, Alu.add)
                for h in range(G):
                    psk = psum.tile([P, 512], dt.float32, tag="mm")
                    for ht in range(HT):
                        nc.tensor.matmul(psk,
                                         wk_sb[:, ht, h * HD:(h + 1) * HD],
                                         xt_sb[:, ht, :],
                                         start=(ht == 0), stop=(ht == HT - 1))
                    nc.vector.tensor_scalar_add(kt_sb[:, h, sl], psk,
                                                bk_sb[:, h:h + 1])
                for sv in range(XC // P):
                    sm = xc * (XC // P) + sv
                    psv = psum.tile([P, 512], dt.float32, tag="mm")
                    for ht in range(HT):
                        nc.tensor.matmul(psv,
                                         xt_sb[:, ht, sv * P:(sv + 1) * P],
                                         wv_sb[:, ht, :],
                                         start=(ht == 0), stop=(ht == HT - 1))
                    nc.vector.tensor_copy(out=v_sb[:, sm, :], in_=psv)

            # out-proj weights: needed only from the first proj (~mid-kernel)
            wo_sb = wpool.tile([P, G, H], dt.bfloat16)
            nc.sync.dma_start(wo_sb[:], wot_r)

            # -------- Phase 2+3: attention + out-proj (sw-pipelined) --------
            def emit_st_exp(h, qc):
                # ST^T = k^T.T @ q^T per 128-key tile; exp on ACT in 2-bank
                # batches (halves the 352-cycle per-ACTIVATE overhead).
                e_sb = epool.tile([P, KT, QC], dt.bfloat16, tag="e",
                                  name="e_sb")
                for km in range(0, KT, 2):
                    ps = psum.tile([P, 2, QC], dt.float32, tag="st")
                    for j in range(2):
                        nc.tensor.matmul(ps[:, j, :],
                                         kt_sb[:, h, (km + j) * P:(km + j + 1) * P],
                                         qt_sb[:, h, qc * QC:(qc + 1) * QC],
                                         start=True, stop=True)
                    nc.scalar.activation(e_sb[:, km:km + 2, :], ps, Act.Exp)
                return e_sb

            def emit_pv_z_norm(h, qc, e_sb):
                pv = psum.tile([P, QC], dt.float32, tag="pv", bufs=1)
                for km in range(KT):
                    nc.tensor.matmul(pv, v_sb[:, km, h * HD:(h + 1) * HD],
                                     e_sb[:, km, :],
                                     start=(km == 0), stop=(km == KT - 1))
                # softmax denominator: ones-matmul sums over keys (partitions)
                # and lands it already broadcast across all 128 partitions
                z = psum.tile([P, QC], dt.float32, tag="z", bufs=1)
                for km in range(KT):
                    nc.tensor.matmul(z, ones_sb[:], e_sb[:, km, :],
                                     start=(km == 0), stop=(km == KT - 1))
                zi = small.tile([P, QC], dt.float32, tag="zi")
                nc.vector.reciprocal_approx_fast(out=zi[:], in_=z)
                nc.vector.tensor_mul(out=at_sb[:, h, qc * QC:(qc + 1) * QC],
                                     in0=pv, in1=zi[:])

            def emit_proj(qc, last=False):
                for sv in range(QC // P):
                    sm = qc * (QC // P) + sv
                    for oc in range(H // 512):
                        pp = psum.tile([P, 512], dt.float32, tag="mm")
                        for g in range(G):
                            nc.tensor.matmul(pp,
                                             at_sb[:, g, sm * P:(sm + 1) * P],
                                             wo_sb[:, g, oc * 512:(oc + 1) * 512],
                                             start=(g == 0), stop=(g == G - 1))
                        ob = small.tile([P, 512], dt.float32, tag="ob", bufs=3)
                        # in the final group, split the drain copies across
                        # DVE and ACT so the tail isn't serialized on one
                        # engine (Copy is in every ACT table set: no reload)
                        if last and oc % 2 == 1:
                            nc.scalar.copy(ob[:], pp)
                        else:
                            nc.vector.tensor_copy(out=ob[:], in_=pp)
                        nc.sync.dma_start(
                            out_d[sm * P:(sm + 1) * P, oc * 512:(oc + 1) * 512],
                            ob[:])

            chunks = [(h, qc) for qc in range(NQC) for h in range(G)]
            prev = None
            for (h, qc) in chunks:
                e = emit_st_exp(h, qc)
                if prev is not None:
                    ph, pqc, pe = prev
                    emit_pv_z_norm(ph, pqc, pe)
                    if ph == G - 1:
                        emit_proj(pqc)
                prev = (h, qc, e)
            ph, pqc, pe = prev
            emit_pv_z_norm(ph, pqc, pe)
            emit_proj(pqc, last=True)

    nc.compile()
    return nc


def _get_nc():
    if "nc" not in _CACHE:
        _CACHE["nc"] = _build()
    return _CACHE["nc"]


def _make_in_maps(x, w_qkv, b_qkv, w_out):
    bf = ml_dtypes.bfloat16
    f32 = np.float32
    in_maps = []
    for c in range(N_CORES):
        b = c // 4
        g = c % 4
        lo = GH * g
        hi = GH * (g + 1)
        xt = np.ascontiguousarray(x[b].T).astype(bf)
        wqt = np.ascontiguousarray(w_qkv[lo:hi, :].T).astype(bf)
        wkt = np.ascontiguousarray(w_qkv[H + lo:H + hi, :].T).astype(bf)
        wvt = np.ascontiguousarray(w_qkv[2 * H + lo:2 * H + hi, :].T).astype(bf)
        bqs = np.ascontiguousarray(
            (b_qkv[lo:hi] * SCALE).astype(f32).reshape(G, P).T)
        bk = np.ascontiguousarray(
            b_qkv[H + lo:H + hi].astype(f32).reshape(G, P).T)
        wot = np.ascontiguousarray(w_out[:, lo:hi].T).astype(bf)
        in_maps.append({"xt": xt, "wqt": wqt, "wkt": wkt, "wvt": wvt,
                        "bqs": bqs, "bk": bk, "wot": wot})
    return in_maps


def kernel(x, w_qkv, b_qkv, w_out, b_out):
    import os
    import sys

    x = np.asarray(x, dtype=np.float32)
    w_qkv = np.asarray(w_qkv, dtype=np.float32)
    b_qkv = np.asarray(b_qkv, dtype=np.float32)
    w_out = np.asarray(w_out, dtype=np.float32)
    b_out = np.asarray(b_out, dtype=np.float32)

    from concourse.bass_utils import run_bass_kernel_spmd

    # NTFF tracing under axon needs the antenv.axon_hooks shim (test.py
    # installs it); without it a stray BASS_TRACE=1 in the environment would
    # crash the run — disable tracing in that case.
    if "antenv.axon_hooks" not in sys.modules:
        os.environ["BASS_NEVER_TRACE"] = "1"

    nc = _get_nc()
    in_maps = _make_in_maps(x, w_qkv, b_qkv, w_out)
    res = run_bass_kernel_spmd(nc, in_maps, core_ids=list(range(N_CORES)))
    _CACHE["last_results"] = res
    partials = [r["partial"] for r in res.results]

    bv = b_qkv[2 * H:3 * H]
    bias = b_out + w_out @ bv          # folded v-bias contribution
    out = np.empty((B, S, H), np.float32)
    for b in range(B):
        acc = partials[4 * b].copy()
        for g in range(1, 4):
            acc += partials[4 * b + g]
        out[b] = acc + bias
    return out



# revision 4
# speedup vs baseline: 1.1537x; 1.1537x over previous
"""Trainium2 Bass kernel for NoTPAttention (dense transformer block:
fused QKV projection -> multi-head attention -> output projection).

Sharding (8 NeuronCores): core c handles batch b = c // 4 and the 4 heads
g = 4*(c % 4) .. 4*(c % 4)+3 (head-parallel tensor parallelism).  Each core
computes its heads' partial out-projection [S, H] in bf16; the host sums the
4 partials per batch in fp32 and adds the (folded) biases.

Numerics: all matmuls run in bf16 with fp32 PSUM accumulation.  Softmax is
computed without max-subtraction (scores are bounded, |s| < ~3.5) with the
normalization deferred to the attention *output*:
    attnT[d, q] = (sum_k v[k, d] * exp(sT[k, q])) / (sum_k exp(sT[k, q]))
The denominator is computed cheaply: the DVE pre-reduces the 16 key-tiles of
exp(sT) with a 4-level tree of bf16 adds ([128,16,512] -> [128,512]), and a
SINGLE ones-matmul per chunk does the remaining 128-partition sum, landing
the result already broadcast across partitions (16x less tensor-engine work
than ones-matmul-ing the full exp tensor).  The v-bias is dropped in-kernel:
after normalization it contributes exactly b_v to every row, so the host
folds w_out @ b_v into the output bias.

Pipeline: phase 2/3 run as 16 uniform iterations, each emitting (on PE):
  z-matmul(i-1) | ST pair 0,1 of chunk i+1 | PV(i) | 4 out-proj groups of
  the previous qc | ST pairs 2-7 of chunk i+1
~10.4us of tensor work per iteration vs ~9.2us of ACT exp, so the scalar
engine (1 elem/cycle/lane @1.2GHz, the hard exp floor) never becomes the
critical path.  Out-proj PSUM groups and the z matmul share the phase-1
"mm" PSUM tag so the total stays exactly 8 banks.  ST(0)/ST(1) are
interleaved into phase 1's last v-projections so exp warms up early.

Layout notes: qT/kT/attnT live as [128 (head-dim), head, seq] so every
matmul contracts over a full 128-partition tile with no transposes anywhere.
The qkv weights share SBUF slots with the attention exp-buffers (tag "e"):
they are dead once the projections finish, exactly when the exp buffers
start rotating.
"""

import numpy as np
import ml_dtypes

B, S, H = 2, 2048, 2048
NH, HD = 16, 128
P = 128
HT = H // P            # 16 hidden-dim tiles
G = 4                  # heads per core
GH = G * HD            # 512: head-group width per core
SCALE = 1.0 / float(np.sqrt(HD))
N_CORES = 8
XC = 512               # phase-1 x streaming chunk (s elements)
QC = 512               # attention query chunk
KT = S // P            # 16 key tiles

_CACHE = {}


def _build():
    import concourse.mybir as mybir
    import concourse.tile as tile
    from concourse import bacc

    dt = mybir.dt
    Alu = mybir.AluOpType
    Act = mybir.ActivationFunctionType

    nc = bacc.Bacc("TRN2", target_bir_lowering=False, debug=False,
                   enable_asserts=False)

    xt_d = nc.dram_tensor("xt", [H, S], dt.bfloat16, kind="ExternalInput").ap()
    wqt_d = nc.dram_tensor("wqt", [H, GH], dt.bfloat16, kind="ExternalInput").ap()
    wkt_d = nc.dram_tensor("wkt", [H, GH], dt.bfloat16, kind="ExternalInput").ap()
    wvt_d = nc.dram_tensor("wvt", [H, GH], dt.bfloat16, kind="ExternalInput").ap()
    bqs_d = nc.dram_tensor("bqs", [P, G], dt.float32, kind="ExternalInput").ap()
    bk_d = nc.dram_tensor("bk", [P, G], dt.float32, kind="ExternalInput").ap()
    wot_d = nc.dram_tensor("wot", [GH, H], dt.bfloat16, kind="ExternalInput").ap()
    out_d = nc.dram_tensor("partial", [S, H], dt.bfloat16,
                           kind="ExternalOutput").ap()

    xt_r = xt_d.rearrange("(ht p) s -> p ht s", p=P)      # [128, 16, 2048]
    wqt_r = wqt_d.rearrange("(ht p) o -> p ht o", p=P)    # [128, 16, 512]
    wkt_r = wkt_d.rearrange("(ht p) o -> p ht o", p=P)
    wvt_r = wvt_d.rearrange("(ht p) o -> p ht o", p=P)
    wot_r = wot_d.rearrange("(g p) o -> p g o", p=P)      # [128, 4, 2048]

    NXC = S // XC      # 4
    NQC = S // QC      # 4

    with tile.TileContext(nc) as tc:
        with (
            tc.tile_pool(name="consts", bufs=1) as consts,
            tc.tile_pool(name="wpool", bufs=1) as wpool,
            tc.tile_pool(name="xpool", bufs=2) as xpool,
            tc.tile_pool(name="big", bufs=1) as big,
            tc.tile_pool(name="epool", bufs=3) as epool,
            tc.tile_pool(name="tree", bufs=1) as tpool,
            tc.tile_pool(name="espool", bufs=2) as espool,
            tc.tile_pool(name="small", bufs=2) as small,
            tc.tile_pool(name="psum", bufs=2, space="PSUM") as psum,
        ):
            # --- startup DMAs: finest-grained interleave of the wq and xt
            # slices the very first accumulation group needs, so the first
            # matmul can start after ~1MB of traffic instead of ~3MB ---
            wq_sb = epool.tile([P, HT, GH], dt.bfloat16, tag="e", name="wq_sb")
            xt0_sb = xpool.tile([P, HT, XC], dt.bfloat16, tag="xt",
                                name="xt0_sb")
            for b4 in range(4):
                hs = slice(4 * b4, 4 * (b4 + 1))
                nc.sync.dma_start(wq_sb[:, hs, :], wqt_r[:, hs, :])
                nc.sync.dma_start(xt0_sb[:, hs, :], xt_r[:, hs, 0:XC])
            bqs_sb = consts.tile([P, G], dt.float32)
            nc.sync.dma_start(bqs_sb[:], bqs_d)
            bk_sb = consts.tile([P, G], dt.float32)
            nc.sync.dma_start(bk_sb[:], bk_d)
            ones_sb = consts.tile([P, P], dt.bfloat16)
            nc.vector.memset(ones_sb[:], 1.0)
            wk_sb = epool.tile([P, HT, GH], dt.bfloat16, tag="e", name="wk_sb")
            nc.sync.dma_start(wk_sb[:], wkt_r)
            wv_sb = epool.tile([P, HT, GH], dt.bfloat16, tag="e", name="wv_sb")
            nc.sync.dma_start(wv_sb[:], wvt_r)

            qt_sb = big.tile([P, G, S], dt.bfloat16)   # q^T, scale+bias applied
            kt_sb = big.tile([P, G, S], dt.bfloat16)   # k^T, bias applied
            v_sb = big.tile([P, KT, GH], dt.bfloat16)  # v natural [s, o]
            at_sb = big.tile([P, G, S], dt.bfloat16)   # attn output^T

            chunks = [(h, qc) for qc in range(NQC) for h in range(G)]
            NCH = len(chunks)

            # ---------- phase 2 emit helpers (defined early: ST(0) is ----
            # ---------- interleaved into phase 1's last v-projections) ----
            e_tiles = {}
            es_tiles = {}
            pv_tiles = {}
            zi_tiles = {}

            def emit_st_pair(i, km):
                # ST^T = k^T.T @ q^T for key tiles km, km+1; exp on ACT in a
                # 2-bank batch (halves the 352-cycle per-ACTIVATE overhead).
                h, qc = chunks[i]
                if km == 0:
                    e_tiles[i] = epool.tile([P, KT, QC], dt.bfloat16, tag="e",
                                            name="e_sb")
                e_sb = e_tiles[i]
                ps = psum.tile([P, 2, QC], dt.float32, tag="st")
                for j in range(2):
                    nc.tensor.matmul(ps[:, j, :],
                                     kt_sb[:, h, (km + j) * P:(km + j + 1) * P],
                                     qt_sb[:, h, qc * QC:(qc + 1) * QC],
                                     start=True, stop=True)
                nc.scalar.activation(e_sb[:, km:km + 2, :], ps, Act.Exp)

            def emit_pv(i):
                h, qc = chunks[i]
                pv = psum.tile([P, QC], dt.float32, tag="pv")
                for km in range(KT):
                    nc.tensor.matmul(pv, v_sb[:, km, h * HD:(h + 1) * HD],
                                     e_tiles[i][:, km, :],
                                     start=(km == 0), stop=(km == KT - 1))
                pv_tiles[i] = pv

            def emit_tree(i):
                # KT-axis pre-reduction of exp(sT) on the DVE: 4 levels of
                # contiguous bf16 adds, [128,16,512] -> [128,512].
                e_sb = e_tiles[i]
                t1 = tpool.tile([P, 8, QC], dt.bfloat16, tag="t1")
                t2 = tpool.tile([P, 4, QC], dt.bfloat16, tag="t2")
                t3 = tpool.tile([P, 2, QC], dt.bfloat16, tag="t3")
                es = espool.tile([P, QC], dt.bfloat16, tag="es", name="es_sb")
                nc.vector.tensor_add(t1[:], e_sb[:, 0:8, :], e_sb[:, 8:16, :])
                nc.vector.tensor_add(t2[:], t1[:, 0:4, :], t1[:, 4:8, :])
                nc.vector.tensor_add(t3[:], t2[:, 0:2, :], t2[:, 2:4, :])
                nc.vector.tensor_add(es[:], t3[:, 0, :], t3[:, 1, :])
                es_tiles[i] = es

            def emit_znorm(i):
                # single ones-matmul finishes the softmax denominator: sums
                # the 128 partitions of es and lands z broadcast in PSUM.
                h, qc = chunks[i]
                z = psum.tile([P, QC], dt.float32, tag="mm")
                nc.tensor.matmul(z, ones_sb[:], es_tiles[i][:],
                                 start=True, stop=True)
                zi = small.tile([P, QC], dt.float32, tag="zi")
                nc.vector.reciprocal_approx_fast(out=zi[:], in_=z)
                nc.vector.tensor_mul(out=at_sb[:, h, qc * QC:(qc + 1) * QC],
                                     in0=pv_tiles[i], in1=zi[:])

            def emit_proj_group(qc, grp, last=False):
                # one out-proj PSUM group: accumulate the 4 heads for one
                # (seq-tile, out-col) block and drain it.
                sv, oc = grp // 4, grp % 4
                sm = qc * (QC // P) + sv
                pp = psum.tile([P, 512], dt.float32, tag="mm")
                for g in range(G):
                    nc.tensor.matmul(pp,
                                     at_sb[:, g, sm * P:(sm + 1) * P],
                                     wo_sb[:, g, oc * 512:(oc + 1) * 512],
                                     start=(g == 0), stop=(g == G - 1))
                ob = small.tile([P, 512], dt.bfloat16, tag="ob", bufs=3)
                # in the final (post-pipeline) groups ACT is idle: split the
                # drain copies across DVE and ACT so the tail isn't
                # serialized on one engine.
                if last and grp % 2 == 1:
                    nc.scalar.copy(ob[:], pp)
                else:
                    nc.vector.tensor_copy(out=ob[:], in_=pp)
                nc.sync.dma_start(
                    out_d[sm * P:(sm + 1) * P, oc * 512:(oc + 1) * 512],
                    ob[:])

            # ---------------- Phase 1: QKV projections ----------------
            for xc in range(NXC):
                if xc == 0:
                    xt_sb = xt0_sb
                else:
                    xt_sb = xpool.tile([P, HT, XC], dt.bfloat16, tag="xt",
                                       name="xt_sb")
                    nc.sync.dma_start(xt_sb[:], xt_r[:, :, xc * XC:(xc + 1) * XC])
                sl = slice(xc * XC, (xc + 1) * XC)
                for h in range(G):
                    psq = psum.tile([P, 512], dt.float32, tag="mm")
                    for ht in range(HT):
                        nc.tensor.matmul(psq,
                                         wq_sb[:, ht, h * HD:(h + 1) * HD],
                                         xt_sb[:, ht, :],
                                         start=(ht == 0), stop=(ht == HT - 1))
                    nc.vector.tensor_scalar(qt_sb[:, h, sl], psq,
                                            SCALE, bqs_sb[:, h:h + 1],
                                            Alu.mult, Alu.add)
                for h in range(G):
                    psk = psum.tile([P, 512], dt.float32, tag="mm")
                    for ht in range(HT):
                        nc.tensor.matmul(psk,
                                         wk_sb[:, ht, h * HD:(h + 1) * HD],
                                         xt_sb[:, ht, :],
                                         start=(ht == 0), stop=(ht == HT - 1))
                    nc.vector.tensor_scalar_add(kt_sb[:, h, sl], psk,
                                                bk_sb[:, h:h + 1])
                for sv in range(XC // P):
                    sm = xc * (XC // P) + sv
                    psv = psum.tile([P, 512], dt.float32, tag="mm")
                    for ht in range(HT):
                        nc.tensor.matmul(psv,
                                         xt_sb[:, ht, sv * P:(sv + 1) * P],
                                         wv_sb[:, ht, :],
                                         start=(ht == 0), stop=(ht == HT - 1))
                    nc.vector.tensor_copy(out=v_sb[:, sm, :], in_=psv)
                    # interleave ST(0)+ST(1) into the last v-projections:
                    # qt/kt of heads 0/1 are complete once xc3's k-projs are
                    # done, so exp warms up ~14us early under the v work and
                    # the qc0 iterations (which have no out-proj filler)
                    # start with ACT ahead instead of behind.
                    if xc == NXC - 1:
                        for km in range(8 * sv, 8 * sv + 8, 2):
                            emit_st_pair(km // 16, km % 16)

            # out-proj weights: needed only from the first proj (~mid-kernel)
            wo_sb = wpool.tile([P, G, H], dt.bfloat16)
            nc.sync.dma_start(wo_sb[:], wot_r)

            # -------- Phase 2+3: attention + out-proj, uniform pipeline ----
            for i in range(NCH):
                h, qc = chunks[i]
                if i >= 1:
                    emit_znorm(i - 1)
                if 1 <= i < NCH - 1:
                    emit_st_pair(i + 1, 0)
                    emit_st_pair(i + 1, 2)
                emit_pv(i)
                # spread the previous qc's out-proj over this qc's 4
                # iterations (4 PSUM groups each); the DVE drain copies are
                # interleaved around the tree so the shared "mm" PSUM
                # rotation never blocks the tensor engine.
                pgs = list(range(4 * h, 4 * h + 4)) if qc >= 1 else []
                for grp in pgs[:2]:
                    emit_proj_group(qc - 1, grp)
                emit_tree(i)
                for grp in pgs[2:]:
                    emit_proj_group(qc - 1, grp)
                if 1 <= i < NCH - 1:
                    for km in range(4, KT, 2):
                        emit_st_pair(i + 1, km)
            emit_znorm(NCH - 1)
            for grp in range(16):
                emit_proj_group(NQC - 1, grp, last=True)

    nc.compile()
    return nc


def _get_nc():
    if "nc" not in _CACHE:
        _CACHE["nc"] = _build()
    return _CACHE["nc"]


def _make_in_maps(x, w_qkv, b_qkv, w_out):
    bf = ml_dtypes.bfloat16
    f32 = np.float32
    in_maps = []
    for c in range(N_CORES):
        b = c // 4
        g = c % 4
        lo = GH * g
        hi = GH * (g + 1)
        xt = np.ascontiguousarray(x[b].T).astype(bf)
        wqt = np.ascontiguousarray(w_qkv[lo:hi, :].T).astype(bf)
        wkt = np.ascontiguousarray(w_qkv[H + lo:H + hi, :].T).astype(bf)
        wvt = np.ascontiguousarray(w_qkv[2 * H + lo:2 * H + hi, :].T).astype(bf)
        bqs = np.ascontiguousarray(
            (b_qkv[lo:hi] * SCALE).astype(f32).reshape(G, P).T)
        bk = np.ascontiguousarray(
            b_qkv[H + lo:H + hi].astype(f32).reshape(G, P).T)
        wot = np.ascontiguousarray(w_out[:, lo:hi].T).astype(bf)
        in_maps.append({"xt": xt, "wqt": wqt, "wkt": wkt, "wvt": wvt,
                        "bqs": bqs, "bk": bk, "wot": wot})
    return in_maps


def kernel(x, w_qkv, b_qkv, w_out, b_out):
    import os
    import sys

    x = np.asarray(x, dtype=np.float32)
    w_qkv = np.asarray(w_qkv, dtype=np.float32)
    b_qkv = np.asarray(b_qkv, dtype=np.float32)
    w_out = np.asarray(w_out, dtype=np.float32)
    b_out = np.asarray(b_out, dtype=np.float32)

    from concourse.bass_utils import run_bass_kernel_spmd

    # NTFF tracing under axon needs the antenv.axon_hooks shim (test.py
    # installs it); without it a stray BASS_TRACE=1 in the environment would
    # crash the run — disable tracing in that case.
    if "antenv.axon_hooks" not in sys.modules:
        os.environ["BASS_NEVER_TRACE"] = "1"

    nc = _get_nc()
    in_maps = _make_in_maps(x, w_qkv, b_qkv, w_out)
    res = run_bass_kernel_spmd(nc, in_maps, core_ids=list(range(N_CORES)))
    _CACHE["last_results"] = res
    partials = [r["partial"] for r in res.results]

    bv = b_qkv[2 * H:3 * H]
    bias = b_out + w_out @ bv          # folded v-bias contribution
    out = np.empty((B, S, H), np.float32)
    for b in range(B):
        acc = partials[4 * b].astype(np.float32)
        for g in range(1, 4):
            acc += partials[4 * b + g].astype(np.float32)
        out[b] = acc + bias
    return out


# revision 8
# speedup vs baseline: 1.1584x; 1.0041x over previous
"""Trainium2 Bass kernel for NoTPAttention (dense transformer block:
fused QKV projection -> multi-head attention -> output projection).

Sharding (8 NeuronCores): core c handles batch b = c // 4 and the 4 heads
g = 4*(c % 4) .. 4*(c % 4)+3 (head-parallel tensor parallelism).  Each core
computes its heads' partial out-projection [S, H] in bf16; the host sums the
4 partials per batch in fp32 and adds the (folded) biases.

Numerics: all matmuls run in bf16 with fp32 PSUM accumulation.  Softmax is
computed without max-subtraction (scores are bounded, |s| < ~3.5) with the
normalization deferred to the attention *output*:
    attnT[d, q] = (sum_k v[k, d] * exp(sT[k, q])) / (sum_k exp(sT[k, q]))
The denominator is computed cheaply: the DVE pre-reduces the 16 key-tiles of
exp(sT) with a 4-level tree of bf16 adds ([128,16,512] -> [128,512]), and a
SINGLE ones-matmul per chunk does the remaining 128-partition sum, landing
the result already broadcast across partitions (16x less tensor-engine work
than ones-matmul-ing the full exp tensor).  The v-bias is dropped in-kernel:
after normalization it contributes exactly b_v to every row, so the host
folds w_out @ b_v into the output bias.

Pipeline: phase 2/3 run as 16 uniform iterations, each emitting (on PE):
  z-matmul(i-1) | ST pair 0,1 of chunk i+1 | PV(i) | 4 out-proj groups of
  the previous qc | ST pairs 2-7 of chunk i+1
~10.4us of tensor work per iteration vs ~9.2us of ACT exp, so the scalar
engine (1 elem/cycle/lane @1.2GHz, the hard exp floor) never becomes the
critical path.  Out-proj PSUM groups and the z matmul share the phase-1
"mm" PSUM tag so the total stays exactly 8 banks.  ST(0)/ST(1) are
interleaved into phase 1's last v-projections so exp warms up early.

Layout notes: qT/kT/attnT live as [128 (head-dim), head, seq] so every
matmul contracts over a full 128-partition tile with no transposes anywhere.
The qkv weights share SBUF slots with the attention exp-buffers (tag "e"):
they are dead once the projections finish, exactly when the exp buffers
start rotating.
"""

import numpy as np
import ml_dtypes

B, S, H = 2, 2048, 2048
NH, HD = 16, 128
P = 128
HT = H // P            # 16 hidden-dim tiles
G = 4                  # heads per core
GH = G * HD            # 512: head-group width per core
SCALE = 1.0 / float(np.sqrt(HD))
N_CORES = 8
XC = 512               # phase-1 x streaming chunk (s elements)
QC = 512               # attention query chunk
KT = S // P            # 16 key tiles

_CACHE = {}


def _build():
    import concourse.mybir as mybir
    import concourse.tile as tile
    from concourse import bacc

    dt = mybir.dt
    Alu = mybir.AluOpType
    Act = mybir.ActivationFunctionType

    nc = bacc.Bacc("TRN2", target_bir_lowering=False, debug=False,
                   enable_asserts=False)

    xt_d = nc.dram_tensor("xt", [H, S], dt.bfloat16, kind="ExternalInput").ap()
    wqt_d = nc.dram_tensor("wqt", [H, GH], dt.bfloat16, kind="ExternalInput").ap()
    wkt_d = nc.dram_tensor("wkt", [H, GH], dt.bfloat16, kind="ExternalInput").ap()
    wvt_d = nc.dram_tensor("wvt", [H, GH], dt.bfloat16, kind="ExternalInput").ap()
    bqs_d = nc.dram_tensor("bqs", [P, G], dt.float32, kind="ExternalInput").ap()
    bk_d = nc.dram_tensor("bk", [P, G], dt.float32, kind="ExternalInput").ap()
    wot_d = nc.dram_tensor("wot", [GH, H], dt.bfloat16, kind="ExternalInput").ap()
    out_d = nc.dram_tensor("partial", [S, H], dt.bfloat16,
                           kind="ExternalOutput").ap()

    xt_r = xt_d.rearrange("(ht p) s -> p ht s", p=P)      # [128, 16, 2048]
    wqt_r = wqt_d.rearrange("(ht p) o -> p ht o", p=P)    # [128, 16, 512]
    wkt_r = wkt_d.rearrange("(ht p) o -> p ht o", p=P)
    wvt_r = wvt_d.rearrange("(ht p) o -> p ht o", p=P)
    wot_r = wot_d.rearrange("(g p) o -> p g o", p=P)      # [128, 4, 2048]

    NXC = S // XC      # 4
    NQC = S // QC      # 4

    with tile.TileContext(nc) as tc:
        with (
            tc.tile_pool(name="consts", bufs=1) as consts,
            tc.tile_pool(name="wpool", bufs=1) as wpool,
            tc.tile_pool(name="xpool", bufs=2) as xpool,
            tc.tile_pool(name="big", bufs=1) as big,
            tc.tile_pool(name="epool", bufs=3) as epool,
            tc.tile_pool(name="tree", bufs=1) as tpool,
            tc.tile_pool(name="espool", bufs=2) as espool,
            tc.tile_pool(name="small", bufs=2) as small,
            tc.tile_pool(name="psum", bufs=2, space="PSUM") as psum,
        ):
            # --- startup DMAs: finest-grained interleave of the wq and xt
            # slices the very first accumulation group needs, so the first
            # matmul can start after ~1MB of traffic instead of ~3MB ---
            wq_sb = epool.tile([P, HT, GH], dt.bfloat16, tag="e", name="wq_sb")
            xt0_sb = xpool.tile([P, HT, XC], dt.bfloat16, tag="xt",
                                name="xt0_sb")
            # 2-ht granules first so the very first accumulation matmuls can
            # start on ~0.5MB of traffic; coarser granules after.
            for hs in [slice(0, 2), slice(2, 4), slice(4, 8),
                       slice(8, 12), slice(12, 16)]:
                nc.sync.dma_start(wq_sb[:, hs, :], wqt_r[:, hs, :])
                nc.sync.dma_start(xt0_sb[:, hs, :], xt_r[:, hs, 0:XC])
            wk_sb = epool.tile([P, HT, GH], dt.bfloat16, tag="e", name="wk_sb")
            wv_sb = epool.tile([P, HT, GH], dt.bfloat16, tag="e", name="wv_sb")
            nc.sync.dma_start(wk_sb[:, 0:4, :], wkt_r[:, 0:4, :])
            bqs_sb = consts.tile([P, G], dt.float32)
            nc.sync.dma_start(bqs_sb[:], bqs_d)
            bk_sb = consts.tile([P, G], dt.float32)
            nc.sync.dma_start(bk_sb[:], bk_d)
            ones_sb = consts.tile([P, P], dt.bfloat16)
            nc.vector.memset(ones_sb[:], 1.0)
            for b4 in range(1, 4):
                hs = slice(4 * b4, 4 * (b4 + 1))
                nc.sync.dma_start(wk_sb[:, hs, :], wkt_r[:, hs, :])
            for b4 in range(4):
                hs = slice(4 * b4, 4 * (b4 + 1))
                nc.sync.dma_start(wv_sb[:, hs, :], wvt_r[:, hs, :])

            qt_sb = big.tile([P, G, S], dt.bfloat16)   # q^T, scale+bias applied
            kt_sb = big.tile([P, G, S], dt.bfloat16)   # k^T, bias applied
            v_sb = big.tile([P, KT, GH], dt.bfloat16)  # v natural [s, o]
            at_sb = big.tile([P, G, S], dt.bfloat16)   # attn output^T

            chunks = [(h, qc) for qc in range(NQC) for h in range(G)]
            NCH = len(chunks)

            # ---------- phase 2 emit helpers (defined early: ST(0) is ----
            # ---------- interleaved into phase 1's last v-projections) ----
            e_tiles = {}
            es_tiles = {}
            pv_tiles = {}
            zi_tiles = {}

            def emit_st_pair(i, km):
                # ST^T = k^T.T @ q^T for key tiles km, km+1; exp on ACT in a
                # 2-bank batch (halves the 352-cycle per-ACTIVATE overhead).
                h, qc = chunks[i]
                if km == 0:
                    e_tiles[i] = epool.tile([P, KT, QC], dt.bfloat16, tag="e",
                                            name="e_sb")
                e_sb = e_tiles[i]
                ps = psum.tile([P, 2, QC], dt.float32, tag="st")
                for j in range(2):
                    nc.tensor.matmul(ps[:, j, :],
                                     kt_sb[:, h, (km + j) * P:(km + j + 1) * P],
                                     qt_sb[:, h, qc * QC:(qc + 1) * QC],
                                     start=True, stop=True)
                nc.scalar.activation(e_sb[:, km:km + 2, :], ps, Act.Exp)

            def emit_pv(i):
                h, qc = chunks[i]
                pv = psum.tile([P, QC], dt.float32, tag="pv")
                for km in range(KT):
                    nc.tensor.matmul(pv, v_sb[:, km, h * HD:(h + 1) * HD],
                                     e_tiles[i][:, km, :],
                                     start=(km == 0), stop=(km == KT - 1))
                pv_tiles[i] = pv

            def emit_tree(i):
                # KT-axis pre-reduction of exp(sT) on the DVE: 4 levels of
                # contiguous bf16 adds, [128,16,512] -> [128,512].
                e_sb = e_tiles[i]
                t1 = tpool.tile([P, 8, QC], dt.bfloat16, tag="t1")
                t2 = tpool.tile([P, 4, QC], dt.bfloat16, tag="t2")
                t3 = tpool.tile([P, 2, QC], dt.bfloat16, tag="t3")
                es = espool.tile([P, QC], dt.bfloat16, tag="es", name="es_sb")
                nc.vector.tensor_add(t1[:], e_sb[:, 0:8, :], e_sb[:, 8:16, :])
                nc.vector.tensor_add(t2[:], t1[:, 0:4, :], t1[:, 4:8, :])
                nc.vector.tensor_add(t3[:], t2[:, 0:2, :], t2[:, 2:4, :])
                nc.vector.tensor_add(es[:], t3[:, 0, :], t3[:, 1, :])
                es_tiles[i] = es

            def emit_tree_incremental(i):
                # last chunk: tree emitted in exp-delivery order so only ~4
                # small adds (not the whole 4us tree) trail the final exp.
                e_sb = e_tiles[i]
                t1 = tpool.tile([P, 8, QC], dt.bfloat16, tag="t1")
                t2 = tpool.tile([P, 4, QC], dt.bfloat16, tag="t2")
                t3 = tpool.tile([P, 2, QC], dt.bfloat16, tag="t3")
                es = espool.tile([P, QC], dt.bfloat16, tag="es", name="es_sb")

                def pair(j):
                    nc.vector.tensor_add(t1[:, j, :],
                                         e_sb[:, 2 * j, :], e_sb[:, 2 * j + 1, :])

                for half in range(2):
                    o = 4 * half
                    pair(o); pair(o + 1)
                    nc.vector.tensor_add(t2[:, o // 2, :],
                                         t1[:, o, :], t1[:, o + 1, :])
                    pair(o + 2); pair(o + 3)
                    nc.vector.tensor_add(t2[:, o // 2 + 1, :],
                                         t1[:, o + 2, :], t1[:, o + 3, :])
                    nc.vector.tensor_add(t3[:, half, :],
                                         t2[:, o // 2, :], t2[:, o // 2 + 1, :])
                nc.vector.tensor_add(es[:], t3[:, 0, :], t3[:, 1, :])
                es_tiles[i] = es

            def emit_znorm(i):
                # single ones-matmul finishes the softmax denominator: sums
                # the 128 partitions of es and lands z broadcast in PSUM.
                h, qc = chunks[i]
                z = psum.tile([P, QC], dt.float32, tag="mm")
                nc.tensor.matmul(z, ones_sb[:], es_tiles[i][:],
                                 start=True, stop=True)
                zi = small.tile([P, QC], dt.float32, tag="zi")
                nc.vector.reciprocal_approx_fast(out=zi[:], in_=z)
                nc.vector.tensor_mul(out=at_sb[:, h, qc * QC:(qc + 1) * QC],
                                     in0=pv_tiles[i], in1=zi[:])

            def emit_proj_group(qc, grp, last=False):
                # one out-proj PSUM group: accumulate the 4 heads for one
                # (seq-tile, out-col) block and drain it.
                sv, oc = grp // 4, grp % 4
                sm = qc * (QC // P) + sv
                pp = psum.tile([P, 512], dt.float32, tag="mm")
                for g in range(G):
                    nc.tensor.matmul(pp,
                                     at_sb[:, g, sm * P:(sm + 1) * P],
                                     wo_sb[:, g, oc * 512:(oc + 1) * 512],
                                     start=(g == 0), stop=(g == G - 1))
                ob = small.tile([P, 512], dt.bfloat16, tag="ob", bufs=3)
                # in the final (post-pipeline) groups ACT is idle: split the
                # drain copies across DVE and ACT so the tail isn't
                # serialized on one engine.
                if last and grp % 2 == 1:
                    nc.scalar.copy(ob[:], pp)
                else:
                    nc.vector.tensor_copy(out=ob[:], in_=pp)
                nc.sync.dma_start(
                    out_d[sm * P:(sm + 1) * P, oc * 512:(oc + 1) * 512],
                    ob[:])

            # ---------------- Phase 1: QKV projections ----------------
            for xc in range(NXC):
                if xc == 0:
                    xt_sb = xt0_sb
                else:
                    xt_sb = xpool.tile([P, HT, XC], dt.bfloat16, tag="xt",
                                       name="xt_sb")
                    nc.sync.dma_start(xt_sb[:], xt_r[:, :, xc * XC:(xc + 1) * XC])
                sl = slice(xc * XC, (xc + 1) * XC)
                for h in range(G):
                    psq = psum.tile([P, 512], dt.float32, tag="mm")
                    for ht in range(HT):
                        nc.tensor.matmul(psq,
                                         wq_sb[:, ht, h * HD:(h + 1) * HD],
                                         xt_sb[:, ht, :],
                                         start=(ht == 0), stop=(ht == HT - 1))
                    nc.vector.tensor_scalar(qt_sb[:, h, sl], psq,
                                            SCALE, bqs_sb[:, h:h + 1],
                                            Alu.mult, Alu.add)
                for h in range(G):
                    psk = psum.tile([P, 512], dt.float32, tag="mm")
                    for ht in range(HT):
                        nc.tensor.matmul(psk,
                                         wk_sb[:, ht, h * HD:(h + 1) * HD],
                                         xt_sb[:, ht, :],
                                         start=(ht == 0), stop=(ht == HT - 1))
                    nc.vector.tensor_scalar_add(kt_sb[:, h, sl], psk,
                                                bk_sb[:, h:h + 1])
                for sv in range(XC // P):
                    sm = xc * (XC // P) + sv
                    psv = psum.tile([P, 512], dt.float32, tag="mm")
                    for ht in range(HT):
                        nc.tensor.matmul(psv,
                                         xt_sb[:, ht, sv * P:(sv + 1) * P],
                                         wv_sb[:, ht, :],
                                         start=(ht == 0), stop=(ht == HT - 1))
                    nc.vector.tensor_copy(out=v_sb[:, sm, :], in_=psv)
                    # interleave ST(0)+ST(1) into the last v-projections:
                    # qt/kt of heads 0/1 are complete once xc3's k-projs are
                    # done, so exp warms up ~14us early under the v work and
                    # the qc0 iterations (which have no out-proj filler)
                    # start with ACT ahead instead of behind.
                    if xc == NXC - 1:
                        for km in range(8 * sv, 8 * sv + 8, 2):
                            emit_st_pair(km // 16, km % 16)

            # out-proj weights: needed only from the first proj (~mid-kernel)
            wo_sb = wpool.tile([P, G, H], dt.bfloat16)
            nc.sync.dma_start(wo_sb[:], wot_r)

            # -------- Phase 2+3: attention + out-proj, uniform pipeline ----
            for i in range(NCH):
                h, qc = chunks[i]
                if i >= 1:
                    emit_znorm(i - 1)
                if 1 <= i < NCH - 1:
                    emit_st_pair(i + 1, 0)
                    emit_st_pair(i + 1, 2)
                emit_pv(i)
                # spread the previous qc's out-proj over this qc's 4
                # iterations (4 PSUM groups each); the DVE drain copies are
                # interleaved around the tree so the shared "mm" PSUM
                # rotation never blocks the tensor engine.
                pgs = list(range(4 * h, 4 * h + 4)) if qc >= 1 else []
                for grp in pgs[:2]:
                    emit_proj_group(qc - 1, grp)
                if i == NCH - 1:
                    for grp in pgs[2:]:
                        emit_proj_group(qc - 1, grp)
                    emit_tree_incremental(i)
                else:
                    emit_tree(i)
                    for grp in pgs[2:]:
                        emit_proj_group(qc - 1, grp)
                if 1 <= i < NCH - 1:
                    for km in range(4, KT, 2):
                        emit_st_pair(i + 1, km)
            emit_znorm(NCH - 1)
            for grp in range(16):
                emit_proj_group(NQC - 1, grp, last=True)

    nc.compile()
    return nc


def _get_nc():
    if "nc" not in _CACHE:
        _CACHE["nc"] = _build()
    return _CACHE["nc"]


def _make_in_maps(x, w_qkv, b_qkv, w_out):
    bf = ml_dtypes.bfloat16
    f32 = np.float32
    in_maps = []
    for c in range(N_CORES):
        b = c // 4
        g = c % 4
        lo = GH * g
        hi = GH * (g + 1)
        xt = np.ascontiguousarray(x[b].T).astype(bf)
        wqt = np.ascontiguousarray(w_qkv[lo:hi, :].T).astype(bf)
        wkt = np.ascontiguousarray(w_qkv[H + lo:H + hi, :].T).astype(bf)
        wvt = np.ascontiguousarray(w_qkv[2 * H + lo:2 * H + hi, :].T).astype(bf)
        bqs = np.ascontiguousarray(
            (b_qkv[lo:hi] * SCALE).astype(f32).reshape(G, P).T)
        bk = np.ascontiguousarray(
            b_qkv[H + lo:H + hi].astype(f32).reshape(G, P).T)
        wot = np.ascontiguousarray(w_out[:, lo:hi].T).astype(bf)
        in_maps.append({"xt": xt, "wqt": wqt, "wkt": wkt, "wvt": wvt,
                        "bqs": bqs, "bk": bk, "wot": wot})
    return in_maps


def kernel(x, w_qkv, b_qkv, w_out, b_out):
    import os
    import sys

    x = np.asarray(x, dtype=np.float32)
    w_qkv = np.asarray(w_qkv, dtype=np.float32)
    b_qkv = np.asarray(b_qkv, dtype=np.float32)
    w_out = np.asarray(w_out, dtype=np.float32)
    b_out = np.asarray(b_out, dtype=np.float32)

    from concourse.bass_utils import run_bass_kernel_spmd

    # NTFF tracing under axon needs the antenv.axon_hooks shim (test.py
    # installs it); without it a stray BASS_TRACE=1 in the environment would
    # crash the run — disable tracing in that case.
    if "antenv.axon_hooks" not in sys.modules:
        os.environ["BASS_NEVER_TRACE"] = "1"

    nc = _get_nc()
    in_maps = _make_in_maps(x, w_qkv, b_qkv, w_out)
    res = run_bass_kernel_spmd(nc, in_maps, core_ids=list(range(N_CORES)))
    _CACHE["last_results"] = res
    partials = [r["partial"] for r in res.results]

    bv = b_qkv[2 * H:3 * H]
    bias = b_out + w_out @ bv          # folded v-bias contribution
    out = np.empty((B, S, H), np.float32)
    for b in range(B):
        acc = partials[4 * b].astype(np.float32)
        for g in range(1, 4):
            acc += partials[4 * b + g].astype(np.float32)
        out[b] = acc + bias
    return out


# revision 11
# speedup vs baseline: 1.1605x; 1.0018x over previous
"""Trainium2 Bass kernel for NoTPAttention (dense transformer block:
fused QKV projection -> multi-head attention -> output projection).

Sharding (8 NeuronCores): core c handles batch b = c // 4 and the 4 heads
g = 4*(c % 4) .. 4*(c % 4)+3 (head-parallel tensor parallelism).  Each core
computes its heads' partial out-projection [S, H] in bf16; the host sums the
4 partials per batch in fp32 and adds the (folded) biases.

Numerics: all matmuls run in bf16 with fp32 PSUM accumulation.  Softmax is
computed without max-subtraction (scores are bounded, |s| < ~3.5) with the
normalization deferred to the attention *output*:
    attnT[d, q] = (sum_k v[k, d] * exp(sT[k, q])) / (sum_k exp(sT[k, q]))
The denominator is computed cheaply: the DVE pre-reduces the 16 key-tiles of
exp(sT) with a 4-level tree of bf16 adds ([128,16,512] -> [128,512]), and a
SINGLE ones-matmul per chunk does the remaining 128-partition sum, landing
the result already broadcast across partitions (16x less tensor-engine work
than ones-matmul-ing the full exp tensor).  The v-bias is dropped in-kernel:
after normalization it contributes exactly b_v to every row, so the host
folds w_out @ b_v into the output bias.

Pipeline: phase 2/3 run as 16 uniform iterations, each emitting (on PE):
  z-matmul(i-1) | ST pair 0,1 of chunk i+1 | PV(i) | 4 out-proj groups of
  the previous qc | ST pairs 2-7 of chunk i+1
~10.4us of tensor work per iteration vs ~9.2us of ACT exp, so the scalar
engine (1 elem/cycle/lane @1.2GHz, the hard exp floor) never becomes the
critical path.  Out-proj PSUM groups and the z matmul share the phase-1
"mm" PSUM tag so the total stays exactly 8 banks.  ST(0)/ST(1) are
interleaved into phase 1's last v-projections so exp warms up early.

Layout notes: qT/kT/attnT live as [128 (head-dim), head, seq] so every
matmul contracts over a full 128-partition tile with no transposes anywhere.
The qkv weights share SBUF slots with the attention exp-buffers (tag "e"):
they are dead once the projections finish, exactly when the exp buffers
start rotating.
"""

import numpy as np
import ml_dtypes

B, S, H = 2, 2048, 2048
NH, HD = 16, 128
P = 128
HT = H // P            # 16 hidden-dim tiles
G = 4                  # heads per core
GH = G * HD            # 512: head-group width per core
SCALE = 1.0 / float(np.sqrt(HD))
N_CORES = 8
XC = 512               # phase-1 x streaming chunk (s elements)
QC = 512               # attention query chunk
KT = S // P            # 16 key tiles

_CACHE = {}


def _build():
    import concourse.mybir as mybir
    import concourse.tile as tile
    from concourse import bacc

    dt = mybir.dt
    Alu = mybir.AluOpType
    Act = mybir.ActivationFunctionType

    nc = bacc.Bacc("TRN2", target_bir_lowering=False, debug=False,
                   enable_asserts=False)

    xt_d = nc.dram_tensor("xt", [H, S], dt.bfloat16, kind="ExternalInput").ap()
    wqt_d = nc.dram_tensor("wqt", [H, GH], dt.bfloat16, kind="ExternalInput").ap()
    wkt_d = nc.dram_tensor("wkt", [H, GH], dt.bfloat16, kind="ExternalInput").ap()
    wvt_d = nc.dram_tensor("wvt", [H, GH], dt.bfloat16, kind="ExternalInput").ap()
    bqs_d = nc.dram_tensor("bqs", [P, G], dt.float32, kind="ExternalInput").ap()
    bk_d = nc.dram_tensor("bk", [P, G], dt.float32, kind="ExternalInput").ap()
    wot_d = nc.dram_tensor("wot", [GH, H], dt.bfloat16, kind="ExternalInput").ap()
    out_d = nc.dram_tensor("partial", [S, H], dt.bfloat16,
                           kind="ExternalOutput").ap()

    xt_r = xt_d.rearrange("(ht p) s -> p ht s", p=P)      # [128, 16, 2048]
    wqt_r = wqt_d.rearrange("(ht p) o -> p ht o", p=P)    # [128, 16, 512]
    wkt_r = wkt_d.rearrange("(ht p) o -> p ht o", p=P)
    wvt_r = wvt_d.rearrange("(ht p) o -> p ht o", p=P)
    wot_r = wot_d.rearrange("(g p) o -> p g o", p=P)      # [128, 4, 2048]

    NXC = S // XC      # 4
    NQC = S // QC      # 4

    with tile.TileContext(nc) as tc:
        with (
            tc.tile_pool(name="consts", bufs=1) as consts,
            tc.tile_pool(name="wpool", bufs=1) as wpool,
            tc.tile_pool(name="xpool", bufs=2) as xpool,
            tc.tile_pool(name="big", bufs=1) as big,
            tc.tile_pool(name="epool", bufs=3) as epool,
            tc.tile_pool(name="tree", bufs=1) as tpool,
            tc.tile_pool(name="espool", bufs=2) as espool,
            tc.tile_pool(name="small", bufs=2) as small,
            tc.tile_pool(name="psum", bufs=2, space="PSUM") as psum,
        ):
            # --- startup DMAs: finest-grained interleave of the wq and xt
            # slices the very first accumulation group needs, so the first
            # matmul can start after ~1MB of traffic instead of ~3MB ---
            wq_sb = epool.tile([P, HT, GH], dt.bfloat16, tag="e", name="wq_sb")
            xt0_sb = xpool.tile([P, HT, XC], dt.bfloat16, tag="xt",
                                name="xt0_sb")
            # 2-ht granules first so the very first accumulation matmuls can
            # start on ~0.5MB of traffic; coarser granules after.
            for hs in [slice(0, 2), slice(2, 4), slice(4, 8),
                       slice(8, 12), slice(12, 16)]:
                nc.sync.dma_start(wq_sb[:, hs, :], wqt_r[:, hs, :])
                nc.sync.dma_start(xt0_sb[:, hs, :], xt_r[:, hs, 0:XC])
            wk_sb = epool.tile([P, HT, GH], dt.bfloat16, tag="e", name="wk_sb")
            wv_sb = epool.tile([P, HT, GH], dt.bfloat16, tag="e", name="wv_sb")
            nc.sync.dma_start(wk_sb[:, 0:4, :], wkt_r[:, 0:4, :])
            bqs_sb = consts.tile([P, G], dt.float32)
            nc.sync.dma_start(bqs_sb[:], bqs_d)
            bk_sb = consts.tile([P, G], dt.float32)
            nc.sync.dma_start(bk_sb[:], bk_d)
            ones_sb = consts.tile([P, P], dt.bfloat16)
            nc.vector.memset(ones_sb[:], 1.0)
            for b4 in range(1, 4):
                hs = slice(4 * b4, 4 * (b4 + 1))
                nc.sync.dma_start(wk_sb[:, hs, :], wkt_r[:, hs, :])
            for b4 in range(4):
                hs = slice(4 * b4, 4 * (b4 + 1))
                nc.sync.dma_start(wv_sb[:, hs, :], wvt_r[:, hs, :])

            qt_sb = big.tile([P, G, S], dt.bfloat16)   # q^T, scale+bias applied
            kt_sb = big.tile([P, G, S], dt.bfloat16)   # k^T, bias applied
            v_sb = big.tile([P, KT, GH], dt.bfloat16)  # v natural [s, o]
            at_sb = big.tile([P, G, S], dt.bfloat16)   # attn output^T

            chunks = [(h, qc) for qc in range(NQC) for h in range(G)]
            NCH = len(chunks)

            # ---------- phase 2 emit helpers (defined early: ST(0) is ----
            # ---------- interleaved into phase 1's last v-projections) ----
            e_tiles = {}
            es_tiles = {}
            pv_tiles = {}
            zi_tiles = {}

            def emit_st_pair(i, km):
                # ST^T = k^T.T @ q^T for key tiles km, km+1; exp on ACT in a
                # 2-bank batch (halves the 352-cycle per-ACTIVATE overhead).
                h, qc = chunks[i]
                if km == 0:
                    e_tiles[i] = epool.tile([P, KT, QC], dt.bfloat16, tag="e",
                                            name="e_sb")
                e_sb = e_tiles[i]
                ps = psum.tile([P, 2, QC], dt.float32, tag="st")
                for j in range(2):
                    nc.tensor.matmul(ps[:, j, :],
                                     kt_sb[:, h, (km + j) * P:(km + j + 1) * P],
                                     qt_sb[:, h, qc * QC:(qc + 1) * QC],
                                     start=True, stop=True)
                nc.scalar.activation(e_sb[:, km:km + 2, :], ps, Act.Exp)

            def emit_pv(i):
                h, qc = chunks[i]
                pv = psum.tile([P, QC], dt.float32, tag="pv")
                for km in range(KT):
                    nc.tensor.matmul(pv, v_sb[:, km, h * HD:(h + 1) * HD],
                                     e_tiles[i][:, km, :],
                                     start=(km == 0), stop=(km == KT - 1))
                pv_tiles[i] = pv

            def emit_tree(i):
                # KT-axis pre-reduction of exp(sT) on the DVE: 4 levels of
                # contiguous bf16 adds, [128,16,512] -> [128,512].
                e_sb = e_tiles[i]
                t1 = tpool.tile([P, 8, QC], dt.bfloat16, tag="t1")
                t2 = tpool.tile([P, 4, QC], dt.bfloat16, tag="t2")
                t3 = tpool.tile([P, 2, QC], dt.bfloat16, tag="t3")
                es = espool.tile([P, QC], dt.bfloat16, tag="es", name="es_sb")
                nc.vector.tensor_add(t1[:], e_sb[:, 0:8, :], e_sb[:, 8:16, :])
                nc.vector.tensor_add(t2[:], t1[:, 0:4, :], t1[:, 4:8, :])
                nc.vector.tensor_add(t3[:], t2[:, 0:2, :], t2[:, 2:4, :])
                nc.vector.tensor_add(es[:], t3[:, 0, :], t3[:, 1, :])
                es_tiles[i] = es

            def emit_tree_incremental(i, interleave=()):
                # last chunk: tree emitted in exp-delivery order so only ~4
                # small adds (not the whole 4us tree) trail the final exp;
                # `interleave` callbacks (the epilogue-feeding drain copies)
                # are sprinkled between the halves so neither blocks the
                # other in the DVE FIFO.
                e_sb = e_tiles[i]
                t1 = tpool.tile([P, 8, QC], dt.bfloat16, tag="t1")
                t2 = tpool.tile([P, 4, QC], dt.bfloat16, tag="t2")
                t3 = tpool.tile([P, 2, QC], dt.bfloat16, tag="t3")
                es = espool.tile([P, QC], dt.bfloat16, tag="es", name="es_sb")
                il = list(interleave)

                def pair(j):
                    nc.vector.tensor_add(t1[:, j, :],
                                         e_sb[:, 2 * j, :], e_sb[:, 2 * j + 1, :])

                for half in range(2):
                    o = 4 * half
                    pair(o); pair(o + 1)
                    nc.vector.tensor_add(t2[:, o // 2, :],
                                         t1[:, o, :], t1[:, o + 1, :])
                    if il:
                        il.pop(0)()
                    pair(o + 2); pair(o + 3)
                    nc.vector.tensor_add(t2[:, o // 2 + 1, :],
                                         t1[:, o + 2, :], t1[:, o + 3, :])
                    nc.vector.tensor_add(t3[:, half, :],
                                         t2[:, o // 2, :], t2[:, o // 2 + 1, :])
                nc.vector.tensor_add(es[:], t3[:, 0, :], t3[:, 1, :])
                for fn in il:
                    fn()
                es_tiles[i] = es

            def emit_znorm(i):
                # single ones-matmul finishes the softmax denominator: sums
                # the 128 partitions of es and lands z broadcast in PSUM.
                h, qc = chunks[i]
                z = psum.tile([P, QC], dt.float32, tag="mm")
                nc.tensor.matmul(z, ones_sb[:], es_tiles[i][:],
                                 start=True, stop=True)
                zi = small.tile([P, QC], dt.float32, tag="zi")
                nc.vector.reciprocal_approx_fast(out=zi[:], in_=z)
                nc.vector.tensor_mul(out=at_sb[:, h, qc * QC:(qc + 1) * QC],
                                     in0=pv_tiles[i], in1=zi[:])

            def emit_proj_group(qc, grp, last=False):
                # one out-proj PSUM group: accumulate the 4 heads for one
                # (seq-tile, out-col) block and drain it.  In the epilogue
                # (last=True) the pv banks are free: alternate tags so the
                # PSUM rotation isn't gated by the 0.7us drain copies.
                sv, oc = grp // 4, grp % 4
                sm = qc * (QC // P) + sv
                tag = ("pv" if grp % 2 else "mm") if last else "mm"
                pp = psum.tile([P, 512], dt.float32, tag=tag)
                for g in range(G):
                    nc.tensor.matmul(pp,
                                     at_sb[:, g, sm * P:(sm + 1) * P],
                                     wo_sb[:, g, oc * 512:(oc + 1) * 512],
                                     start=(g == 0), stop=(g == G - 1))
                ob = small.tile([P, 512], dt.bfloat16, tag="ob", bufs=3)
                # in the final (post-pipeline) groups ACT is idle: split the
                # drain copies across DVE and ACT so the tail isn't
                # serialized on one engine.
                if last and grp % 2 == 1:
                    nc.scalar.copy(ob[:], pp)
                else:
                    nc.vector.tensor_copy(out=ob[:], in_=pp)
                nc.sync.dma_start(
                    out_d[sm * P:(sm + 1) * P, oc * 512:(oc + 1) * 512],
                    ob[:])

            # ---------------- Phase 1: QKV projections ----------------
            for xc in range(NXC):
                if xc == 0:
                    xt_sb = xt0_sb
                else:
                    xt_sb = xpool.tile([P, HT, XC], dt.bfloat16, tag="xt",
                                       name="xt_sb")
                    nc.sync.dma_start(xt_sb[:], xt_r[:, :, xc * XC:(xc + 1) * XC])
                sl = slice(xc * XC, (xc + 1) * XC)
                for h in range(G):
                    psq = psum.tile([P, 512], dt.float32, tag="mm")
                    for ht in range(HT):
                        nc.tensor.matmul(psq,
                                         wq_sb[:, ht, h * HD:(h + 1) * HD],
                                         xt_sb[:, ht, :],
                                         start=(ht == 0), stop=(ht == HT - 1))
                    nc.vector.tensor_scalar(qt_sb[:, h, sl], psq,
                                            SCALE, bqs_sb[:, h:h + 1],
                                            Alu.mult, Alu.add)
                for h in range(G):
                    psk = psum.tile([P, 512], dt.float32, tag="mm")
                    for ht in range(HT):
                        nc.tensor.matmul(psk,
                                         wk_sb[:, ht, h * HD:(h + 1) * HD],
                                         xt_sb[:, ht, :],
                                         start=(ht == 0), stop=(ht == HT - 1))
                    nc.vector.tensor_scalar_add(kt_sb[:, h, sl], psk,
                                                bk_sb[:, h:h + 1])
                for sv in range(XC // P):
                    sm = xc * (XC // P) + sv
                    psv = psum.tile([P, 512], dt.float32, tag="mm")
                    for ht in range(HT):
                        nc.tensor.matmul(psv,
                                         xt_sb[:, ht, sv * P:(sv + 1) * P],
                                         wv_sb[:, ht, :],
                                         start=(ht == 0), stop=(ht == HT - 1))
                    nc.vector.tensor_copy(out=v_sb[:, sm, :], in_=psv)
                    # interleave ST(0)+ST(1) into the last v-projections:
                    # qt/kt of heads 0/1 are complete once xc3's k-projs are
                    # done, so exp warms up ~14us early under the v work and
                    # the qc0 iterations (which have no out-proj filler)
                    # start with ACT ahead instead of behind.
                    if xc == NXC - 1:
                        for km in range(8 * sv, 8 * sv + 8, 2):
                            emit_st_pair(km // 16, km % 16)

            # out-proj weights: needed only from the first proj (~mid-kernel)
            wo_sb = wpool.tile([P, G, H], dt.bfloat16)
            nc.sync.dma_start(wo_sb[:], wot_r)

            # -------- Phase 2+3: attention + out-proj, uniform pipeline ----
            for i in range(NCH):
                h, qc = chunks[i]
                if i >= 1:
                    emit_znorm(i - 1)
                if 1 <= i < NCH - 1:
                    emit_st_pair(i + 1, 0)
                    emit_st_pair(i + 1, 2)
                emit_pv(i)
                # spread the previous qc's out-proj over this qc's 4
                # iterations (4 PSUM groups each); the DVE drain copies are
                # interleaved around the tree so the shared "mm" PSUM
                # rotation never blocks the tensor engine.
                pgs = list(range(4 * h, 4 * h + 4)) if qc >= 1 else []
                if i == NCH - 1:
                    for grp in pgs[:2]:
                        emit_proj_group(qc - 1, grp)
                    emit_tree_incremental(
                        i, [lambda g=g: emit_proj_group(qc - 1, g)
                            for g in pgs[2:]])
                else:
                    for grp in pgs[:2]:
                        emit_proj_group(qc - 1, grp)
                    emit_tree(i)
                    for grp in pgs[2:]:
                        emit_proj_group(qc - 1, grp)
                if 1 <= i < NCH - 1:
                    for km in range(4, KT, 2):
                        emit_st_pair(i + 1, km)
            emit_znorm(NCH - 1)
            for grp in range(16):
                emit_proj_group(NQC - 1, grp, last=True)

    nc.compile()
    return nc


def _get_nc():
    if "nc" not in _CACHE:
        _CACHE["nc"] = _build()
    return _CACHE["nc"]


def _make_in_maps(x, w_qkv, b_qkv, w_out):
    bf = ml_dtypes.bfloat16
    f32 = np.float32
    in_maps = []
    for c in range(N_CORES):
        b = c // 4
        g = c % 4
        lo = GH * g
        hi = GH * (g + 1)
        xt = np.ascontiguousarray(x[b].T).astype(bf)
        wqt = np.ascontiguousarray(w_qkv[lo:hi, :].T).astype(bf)
        wkt = np.ascontiguousarray(w_qkv[H + lo:H + hi, :].T).astype(bf)
        wvt = np.ascontiguousarray(w_qkv[2 * H + lo:2 * H + hi, :].T).astype(bf)
        bqs = np.ascontiguousarray(
            (b_qkv[lo:hi] * SCALE).astype(f32).reshape(G, P).T)
        bk = np.ascontiguousarray(
            b_qkv[H + lo:H + hi].astype(f32).reshape(G, P).T)
        wot = np.ascontiguousarray(w_out[:, lo:hi].T).astype(bf)
        in_maps.append({"xt": xt, "wqt": wqt, "wkt": wkt, "wvt": wvt,
                        "bqs": bqs, "bk": bk, "wot": wot})
    return in_maps


def kernel(x, w_qkv, b_qkv, w_out, b_out):
    import os
    import sys

    x = np.asarray(x, dtype=np.float32)
    w_qkv = np.asarray(w_qkv, dtype=np.float32)
    b_qkv = np.asarray(b_qkv, dtype=np.float32)
    w_out = np.asarray(w_out, dtype=np.float32)
    b_out = np.asarray(b_out, dtype=np.float32)

    from concourse.bass_utils import run_bass_kernel_spmd

    # NTFF tracing under axon needs the antenv.axon_hooks shim (test.py
    # installs it); without it a stray BASS_TRACE=1 in the environment would
    # crash the run — disable tracing in that case.
    if "antenv.axon_hooks" not in sys.modules:
        os.environ["BASS_NEVER_TRACE"] = "1"

    nc = _get_nc()
    in_maps = _make_in_maps(x, w_qkv, b_qkv, w_out)
    res = run_bass_kernel_spmd(nc, in_maps, core_ids=list(range(N_CORES)))
    _CACHE["last_results"] = res
    partials = [r["partial"] for r in res.results]

    bv = b_qkv[2 * H:3 * H]
    bias = b_out + w_out @ bv          # folded v-bias contribution
    out = np.empty((B, S, H), np.float32)
    for b in range(B):
        acc = partials[4 * b].astype(np.float32)
        for g in range(1, 4):
            acc += partials[4 * b + g].astype(np.float32)
        out[b] = acc + bias
    return out


# revision 12
# speedup vs baseline: 1.1703x; 1.0085x over previous
"""Trainium2 Bass kernel for NoTPAttention (dense transformer block:
fused QKV projection -> multi-head attention -> output projection).

Sharding (8 NeuronCores): core c handles batch b = c // 4 and the 4 heads
g = 4*(c % 4) .. 4*(c % 4)+3 (head-parallel tensor parallelism).  Each core
computes its heads' partial out-projection [S, H] in bf16; the host sums the
4 partials per batch in fp32 and adds the (folded) biases.

Numerics: all matmuls run in bf16 with fp32 PSUM accumulation.  Softmax is
computed without max-subtraction (scores are bounded, |s| < ~3.5) with the
normalization deferred to the attention *output*:
    attnT[d, q] = (sum_k v[k, d] * exp(sT[k, q])) / (sum_k exp(sT[k, q]))
The denominator is computed cheaply: the DVE pre-reduces the 16 key-tiles of
exp(sT) with a 4-level tree of bf16 adds ([128,16,512] -> [128,512]), and a
SINGLE ones-matmul per chunk does the remaining 128-partition sum, landing
the result already broadcast across partitions (16x less tensor-engine work
than ones-matmul-ing the full exp tensor).  The v-bias is dropped in-kernel:
after normalization it contributes exactly b_v to every row, so the host
folds w_out @ b_v into the output bias.

Pipeline: phase 2/3 run as 16 uniform iterations, each emitting (on PE):
  z-matmul(i-1) | ST pair 0,1 of chunk i+1 | PV(i) | 4 out-proj groups of
  the previous qc | ST pairs 2-7 of chunk i+1
~10.4us of tensor work per iteration vs ~9.2us of ACT exp, so the scalar
engine (1 elem/cycle/lane @1.2GHz, the hard exp floor) never becomes the
critical path.  Out-proj PSUM groups and the z matmul share the phase-1
"mm" PSUM tag so the total stays exactly 8 banks.  ST(0)/ST(1) are
interleaved into phase 1's last v-projections so exp warms up early.

Layout notes: qT/kT/attnT live as [128 (head-dim), head, seq] so every
matmul contracts over a full 128-partition tile with no transposes anywhere.
The qkv weights share SBUF slots with the attention exp-buffers (tag "e"):
they are dead once the projections finish, exactly when the exp buffers
start rotating.
"""

import numpy as np
import ml_dtypes

B, S, H = 2, 2048, 2048
NH, HD = 16, 128
P = 128
HT = H // P            # 16 hidden-dim tiles
G = 4                  # heads per core
GH = G * HD            # 512: head-group width per core
SCALE = 1.0 / float(np.sqrt(HD))
N_CORES = 8
XC = 512               # phase-1 x streaming chunk (s elements)
QC = 512               # attention query chunk
KT = S // P            # 16 key tiles

_CACHE = {}


def _build():
    import concourse.mybir as mybir
    import concourse.tile as tile
    from concourse import bacc

    dt = mybir.dt
    Alu = mybir.AluOpType
    Act = mybir.ActivationFunctionType

    nc = bacc.Bacc("TRN2", target_bir_lowering=False, debug=False,
                   enable_asserts=False)

    xt_d = nc.dram_tensor("xt", [H, S], dt.bfloat16, kind="ExternalInput").ap()
    wqt_d = nc.dram_tensor("wqt", [H, GH], dt.bfloat16, kind="ExternalInput").ap()
    wkt_d = nc.dram_tensor("wkt", [H, GH], dt.bfloat16, kind="ExternalInput").ap()
    wvt_d = nc.dram_tensor("wvt", [H, GH], dt.bfloat16, kind="ExternalInput").ap()
    bqs_d = nc.dram_tensor("bqs", [P, G], dt.float32, kind="ExternalInput").ap()
    bk_d = nc.dram_tensor("bk", [P, G], dt.float32, kind="ExternalInput").ap()
    wot_d = nc.dram_tensor("wot", [GH, H], dt.bfloat16, kind="ExternalInput").ap()
    out_d = nc.dram_tensor("partial", [S, H], dt.bfloat16,
                           kind="ExternalOutput").ap()

    xt_r = xt_d.rearrange("(ht p) s -> p ht s", p=P)      # [128, 16, 2048]
    wqt_r = wqt_d.rearrange("(ht p) o -> p ht o", p=P)    # [128, 16, 512]
    wkt_r = wkt_d.rearrange("(ht p) o -> p ht o", p=P)
    wvt_r = wvt_d.rearrange("(ht p) o -> p ht o", p=P)
    wot_r = wot_d.rearrange("(g p) o -> p g o", p=P)      # [128, 4, 2048]

    NXC = S // XC      # 4
    NQC = S // QC      # 4

    with tile.TileContext(nc) as tc:
        with (
            tc.tile_pool(name="consts", bufs=1) as consts,
            tc.tile_pool(name="wpool", bufs=1) as wpool,
            tc.tile_pool(name="xpool", bufs=2) as xpool,
            tc.tile_pool(name="big", bufs=1) as big,
            tc.tile_pool(name="epool", bufs=3) as epool,
            tc.tile_pool(name="tree", bufs=1) as tpool,
            tc.tile_pool(name="espool", bufs=2) as espool,
            tc.tile_pool(name="small", bufs=2) as small,
            tc.tile_pool(name="psum", bufs=2, space="PSUM") as psum,
        ):
            # --- startup DMAs: finest-grained interleave of the wq and xt
            # slices the very first accumulation group needs, so the first
            # matmul can start after ~1MB of traffic instead of ~3MB ---
            wq_sb = epool.tile([P, HT, GH], dt.bfloat16, tag="e", name="wq_sb")
            xt0_sb = xpool.tile([P, HT, XC], dt.bfloat16, tag="xt",
                                name="xt0_sb")
            # 2-ht granules first so the very first accumulation matmuls can
            # start on ~0.5MB of traffic; coarser granules after.
            for hs in [slice(0, 2), slice(2, 4), slice(4, 8),
                       slice(8, 12), slice(12, 16)]:
                nc.sync.dma_start(wq_sb[:, hs, :], wqt_r[:, hs, :])
                nc.sync.dma_start(xt0_sb[:, hs, :], xt_r[:, hs, 0:XC])
            wk_sb = epool.tile([P, HT, GH], dt.bfloat16, tag="e", name="wk_sb")
            wv_sb = epool.tile([P, HT, GH], dt.bfloat16, tag="e", name="wv_sb")
            nc.sync.dma_start(wk_sb[:, 0:4, :], wkt_r[:, 0:4, :])
            bqs_sb = consts.tile([P, G], dt.float32)
            nc.sync.dma_start(bqs_sb[:], bqs_d)
            bk_sb = consts.tile([P, G], dt.float32)
            nc.sync.dma_start(bk_sb[:], bk_d)
            ones_sb = consts.tile([P, P], dt.bfloat16)
            nc.vector.memset(ones_sb[:], 1.0)
            for b4 in range(1, 4):
                hs = slice(4 * b4, 4 * (b4 + 1))
                nc.sync.dma_start(wk_sb[:, hs, :], wkt_r[:, hs, :])
            for b4 in range(4):
                hs = slice(4 * b4, 4 * (b4 + 1))
                nc.sync.dma_start(wv_sb[:, hs, :], wvt_r[:, hs, :])

            qt_sb = big.tile([P, G, S], dt.bfloat16)   # q^T, scale+bias applied
            kt_sb = big.tile([P, G, S], dt.bfloat16)   # k^T, bias applied
            v_sb = big.tile([P, KT, GH], dt.bfloat16)  # v natural [s, o]
            at_sb = big.tile([P, G, S], dt.bfloat16)   # attn output^T

            chunks = [(h, qc) for qc in range(NQC) for h in range(G)]
            NCH = len(chunks)

            # ---------- phase 2 emit helpers (defined early: ST(0) is ----
            # ---------- interleaved into phase 1's last v-projections) ----
            e_tiles = {}
            es_tiles = {}
            pv_tiles = {}
            zi_tiles = {}

            def emit_st_pair(i, km):
                # ST^T = k^T.T @ q^T for key tiles km, km+1; exp on ACT in a
                # 2-bank batch (halves the 352-cycle per-ACTIVATE overhead).
                h, qc = chunks[i]
                if km == 0:
                    e_tiles[i] = epool.tile([P, KT, QC], dt.bfloat16, tag="e",
                                            name="e_sb")
                e_sb = e_tiles[i]
                ps = psum.tile([P, 2, QC], dt.float32, tag="st")
                for j in range(2):
                    nc.tensor.matmul(ps[:, j, :],
                                     kt_sb[:, h, (km + j) * P:(km + j + 1) * P],
                                     qt_sb[:, h, qc * QC:(qc + 1) * QC],
                                     start=True, stop=True)
                nc.scalar.activation(e_sb[:, km:km + 2, :], ps, Act.Exp)

            def emit_pv(i):
                h, qc = chunks[i]
                pv = psum.tile([P, QC], dt.float32, tag="pv")
                for km in range(KT):
                    nc.tensor.matmul(pv, v_sb[:, km, h * HD:(h + 1) * HD],
                                     e_tiles[i][:, km, :],
                                     start=(km == 0), stop=(km == KT - 1))
                pv_tiles[i] = pv

            def emit_tree(i):
                # KT-axis pre-reduction of exp(sT) on the DVE: 4 levels of
                # contiguous bf16 adds, [128,16,512] -> [128,512].
                e_sb = e_tiles[i]
                t1 = tpool.tile([P, 8, QC], dt.bfloat16, tag="t1")
                t2 = tpool.tile([P, 4, QC], dt.bfloat16, tag="t2")
                t3 = tpool.tile([P, 2, QC], dt.bfloat16, tag="t3")
                es = espool.tile([P, QC], dt.bfloat16, tag="es", name="es_sb")
                nc.vector.tensor_add(t1[:], e_sb[:, 0:8, :], e_sb[:, 8:16, :])
                nc.vector.tensor_add(t2[:], t1[:, 0:4, :], t1[:, 4:8, :])
                nc.vector.tensor_add(t3[:], t2[:, 0:2, :], t2[:, 2:4, :])
                nc.vector.tensor_add(es[:], t3[:, 0, :], t3[:, 1, :])
                es_tiles[i] = es

            def emit_tree_incremental(i, interleave=()):
                # last chunk: tree emitted in exp-delivery order so only ~4
                # small adds (not the whole 4us tree) trail the final exp;
                # `interleave` callbacks (the epilogue-feeding drain copies)
                # are sprinkled between the halves so neither blocks the
                # other in the DVE FIFO.
                e_sb = e_tiles[i]
                t1 = tpool.tile([P, 8, QC], dt.bfloat16, tag="t1")
                t2 = tpool.tile([P, 4, QC], dt.bfloat16, tag="t2")
                t3 = tpool.tile([P, 2, QC], dt.bfloat16, tag="t3")
                es = espool.tile([P, QC], dt.bfloat16, tag="es", name="es_sb")
                il = list(interleave)

                def pair(j):
                    nc.vector.tensor_add(t1[:, j, :],
                                         e_sb[:, 2 * j, :], e_sb[:, 2 * j + 1, :])

                for half in range(2):
                    o = 4 * half
                    pair(o); pair(o + 1)
                    nc.vector.tensor_add(t2[:, o // 2, :],
                                         t1[:, o, :], t1[:, o + 1, :])
                    if il:
                        il.pop(0)()
                    pair(o + 2); pair(o + 3)
                    nc.vector.tensor_add(t2[:, o // 2 + 1, :],
                                         t1[:, o + 2, :], t1[:, o + 3, :])
                    nc.vector.tensor_add(t3[:, half, :],
                                         t2[:, o // 2, :], t2[:, o // 2 + 1, :])
                nc.vector.tensor_add(es[:], t3[:, 0, :], t3[:, 1, :])
                for fn in il:
                    fn()
                es_tiles[i] = es

            def emit_znorm(i):
                # single ones-matmul finishes the softmax denominator: sums
                # the 128 partitions of es and lands z broadcast in PSUM.
                h, qc = chunks[i]
                z = psum.tile([P, QC], dt.float32, tag="mm")
                nc.tensor.matmul(z, ones_sb[:], es_tiles[i][:],
                                 start=True, stop=True)
                zi = small.tile([P, QC], dt.float32, tag="zi")
                nc.vector.reciprocal_approx_fast(out=zi[:], in_=z)
                nc.vector.tensor_mul(out=at_sb[:, h, qc * QC:(qc + 1) * QC],
                                     in0=pv_tiles[i], in1=zi[:])

            ob_tiles = {}

            def emit_proj_group(qc, grp, last=False):
                # one out-proj PSUM group: accumulate the 4 heads for one
                # (seq-tile, out-col) block and drain it.  In the epilogue
                # (last=True) the pv banks are free: alternate tags so the
                # PSUM rotation isn't gated by the 0.7us drain copies.  The
                # 4 oc-blocks of one seq-tile drain into one ob tile and
                # leave as a single 512KB DMA with 4KB-contiguous rows (1KB
                # packets made the bare final drains take ~8us).
                sv, oc = grp // 4, grp % 4
                sm = qc * (QC // P) + sv
                tag = ("pv" if grp % 2 else "mm") if last else "mm"
                pp = psum.tile([P, 512], dt.float32, tag=tag)
                for g in range(G):
                    nc.tensor.matmul(pp,
                                     at_sb[:, g, sm * P:(sm + 1) * P],
                                     wo_sb[:, g, oc * 512:(oc + 1) * 512],
                                     start=(g == 0), stop=(g == G - 1))
                if oc == 0:
                    ob_tiles[sm] = small.tile([P, G, 512], dt.bfloat16,
                                              tag="ob", name="ob_sb")
                ob = ob_tiles[sm]
                # in the final (post-pipeline) groups ACT is idle: split the
                # drain copies across DVE and ACT so the tail isn't
                # serialized on one engine.
                if last and grp % 2 == 1:
                    nc.scalar.copy(ob[:, oc, :], pp)
                else:
                    nc.vector.tensor_copy(out=ob[:, oc, :], in_=pp)
                if oc == G - 1:
                    nc.sync.dma_start(out_d[sm * P:(sm + 1) * P, :], ob[:])

            # ---------------- Phase 1: QKV projections ----------------
            for xc in range(NXC):
                if xc == 0:
                    xt_sb = xt0_sb
                else:
                    xt_sb = xpool.tile([P, HT, XC], dt.bfloat16, tag="xt",
                                       name="xt_sb")
                    nc.sync.dma_start(xt_sb[:], xt_r[:, :, xc * XC:(xc + 1) * XC])
                sl = slice(xc * XC, (xc + 1) * XC)
                for h in range(G):
                    psq = psum.tile([P, 512], dt.float32, tag="mm")
                    for ht in range(HT):
                        nc.tensor.matmul(psq,
                                         wq_sb[:, ht, h * HD:(h + 1) * HD],
                                         xt_sb[:, ht, :],
                                         start=(ht == 0), stop=(ht == HT - 1))
                    nc.vector.tensor_scalar(qt_sb[:, h, sl], psq,
                                            SCALE, bqs_sb[:, h:h + 1],
                                            Alu.mult, Alu.add)
                for h in range(G):
                    psk = psum.tile([P, 512], dt.float32, tag="mm")
                    for ht in range(HT):
                        nc.tensor.matmul(psk,
                                         wk_sb[:, ht, h * HD:(h + 1) * HD],
                                         xt_sb[:, ht, :],
                                         start=(ht == 0), stop=(ht == HT - 1))
                    nc.vector.tensor_scalar_add(kt_sb[:, h, sl], psk,
                                                bk_sb[:, h:h + 1])
                for sv in range(XC // P):
                    sm = xc * (XC // P) + sv
                    psv = psum.tile([P, 512], dt.float32, tag="mm")
                    for ht in range(HT):
                        nc.tensor.matmul(psv,
                                         xt_sb[:, ht, sv * P:(sv + 1) * P],
                                         wv_sb[:, ht, :],
                                         start=(ht == 0), stop=(ht == HT - 1))
                    nc.vector.tensor_copy(out=v_sb[:, sm, :], in_=psv)
                    # interleave ST(0)+ST(1) into the last v-projections:
                    # qt/kt of heads 0/1 are complete once xc3's k-projs are
                    # done, so exp warms up ~14us early under the v work and
                    # the qc0 iterations (which have no out-proj filler)
                    # start with ACT ahead instead of behind.
                    if xc == NXC - 1:
                        for km in range(8 * sv, 8 * sv + 8, 2):
                            emit_st_pair(km // 16, km % 16)

            # out-proj weights: needed only from the first proj (~mid-kernel)
            wo_sb = wpool.tile([P, G, H], dt.bfloat16)
            nc.sync.dma_start(wo_sb[:], wot_r)

            # -------- Phase 2+3: attention + out-proj, uniform pipeline ----
            for i in range(NCH):
                h, qc = chunks[i]
                if i >= 1:
                    emit_znorm(i - 1)
                if 1 <= i < NCH - 1:
                    emit_st_pair(i + 1, 0)
                    emit_st_pair(i + 1, 2)
                emit_pv(i)
                # spread the previous qc's out-proj over this qc's 4
                # iterations (4 PSUM groups each); the DVE drain copies are
                # interleaved around the tree so the shared "mm" PSUM
                # rotation never blocks the tensor engine.
                pgs = list(range(4 * h, 4 * h + 4)) if qc >= 1 else []
                if i == NCH - 1:
                    for grp in pgs[:2]:
                        emit_proj_group(qc - 1, grp)
                    emit_tree_incremental(
                        i, [lambda g=g: emit_proj_group(qc - 1, g)
                            for g in pgs[2:]])
                else:
                    for grp in pgs[:2]:
                        emit_proj_group(qc - 1, grp)
                    emit_tree(i)
                    for grp in pgs[2:]:
                        emit_proj_group(qc - 1, grp)
                if 1 <= i < NCH - 1:
                    for km in range(4, KT, 2):
                        emit_st_pair(i + 1, km)
            emit_znorm(NCH - 1)
            for grp in range(16):
                emit_proj_group(NQC - 1, grp, last=True)

    nc.compile()
    return nc


def _get_nc():
    if "nc" not in _CACHE:
        _CACHE["nc"] = _build()
    return _CACHE["nc"]


def _make_in_maps(x, w_qkv, b_qkv, w_out):
    bf = ml_dtypes.bfloat16
    f32 = np.float32
    in_maps = []
    for c in range(N_CORES):
        b = c // 4
        g = c % 4
        lo = GH * g
        hi = GH * (g + 1)
        xt = np.ascontiguousarray(x[b].T).astype(bf)
        wqt = np.ascontiguousarray(w_qkv[lo:hi, :].T).astype(bf)
        wkt = np.ascontiguousarray(w_qkv[H + lo:H + hi, :].T).astype(bf)
        wvt = np.ascontiguousarray(w_qkv[2 * H + lo:2 * H + hi, :].T).astype(bf)
        bqs = np.ascontiguousarray(
            (b_qkv[lo:hi] * SCALE).astype(f32).reshape(G, P).T)
        bk = np.ascontiguousarray(
            b_qkv[H + lo:H + hi].astype(f32).reshape(G, P).T)
        wot = np.ascontiguousarray(w_out[:, lo:hi].T).astype(bf)
        in_maps.append({"xt": xt, "wqt": wqt, "wkt": wkt, "wvt": wvt,
                        "bqs": bqs, "bk": bk, "wot": wot})
    return in_maps


def kernel(x, w_qkv, b_qkv, w_out, b_out):
    import os
    import sys

    x = np.asarray(x, dtype=np.float32)
    w_qkv = np.asarray(w_qkv, dtype=np.float32)
    b_qkv = np.asarray(b_qkv, dtype=np.float32)
    w_out = np.asarray(w_out, dtype=np.float32)
    b_out = np.asarray(b_out, dtype=np.float32)

    from concourse.bass_utils import run_bass_kernel_spmd

    # NTFF tracing under axon needs the antenv.axon_hooks shim (test.py
    # installs it); without it a stray BASS_TRACE=1 in the environment would
    # crash the run — disable tracing in that case.
    if "antenv.axon_hooks" not in sys.modules:
        os.environ["BASS_NEVER_TRACE"] = "1"

    nc = _get_nc()
    in_maps = _make_in_maps(x, w_qkv, b_qkv, w_out)
    res = run_bass_kernel_spmd(nc, in_maps, core_ids=list(range(N_CORES)))
    _CACHE["last_results"] = res
    partials = [r["partial"] for r in res.results]

    bv = b_qkv[2 * H:3 * H]
    bias = b_out + w_out @ bv          # folded v-bias contribution
    out = np.empty((B, S, H), np.float32)
    for b in range(B):
        acc = partials[4 * b].astype(np.float32)
        for g in range(1, 4):
            acc += partials[4 * b + g].astype(np.float32)
        out[b] = acc + bias
    return out
